# revision 16
# baseline (speedup 1.0000x reference)
"""Trainium2 Bass kernel for the mixed low-rank-expert DCN-v2 block (nn_DCN_51539607711).

Reference math (L=3 layers, E=4 experts, D=512, R=64, B=16384):
  x_{l+1} = sum_e x0 * (tanh(tanh(x_l V_e) C_e) U_e^T + b_l) * gate_e + x_l
The gate is softmax over a size-1 axis == exactly 1.0, so G never affects the
output. With gate == 1 the recurrence telescopes:
  x_{l+1} = x0 * (1 + sum_{i<=l} (A_i(x_i) + E*b_i)),
  A_i(x) = sum_e U_e tanh(C_e^T tanh(V_e^T x))
so the residual stream is carried as a single PSUM accumulator s = sum_i A_i
(fp32, accumulated by the PE across all experts AND layers), and each layer's
activation update is ONE fused DVE op per chunk:
  x_{l+1} = (s + c_l[d]) * x0,   c_l = 1 + E*cumsum(b)_l   (per-partition scalar)

Distribution: pure data-parallel over B across 8 cores (2048 rows/core),
weights replicated. Activations live feature-major (xT: [D, B]); every matmul
contracts on the partition dim with zero on-device transposes. Experts are
packed in pairs to fill all 128 partitions/output rows:
  v-step :  vT[pair]  = Vpair^T  @ xlT    (lhsT = Vpair [D,128], K=D in 4 chunks)
  cv-step:  cvT[pair] = blockdiag(C_e0,C_e1)^T @ vT[pair]   (K=128)
  ucv    :  s[mchunk] += Upair^T-packed @ cvT[pair]          (K=128, accum)
All matmul operands bf16 (fp32 PSUM accumulation); x0 is kept only in bf16
(rel-err budget is 2e-2; bf16 x0 costs ~3e-3).

Scheduling: the v/cv/tanh chain of block b+1's layer 0 depends only on the x
DMA, so it is emitted as a generator pumped at fill sites inside block b's
layer-1/2 chains. Those fill matmuls occupy the PE exactly where it would
otherwise stall waiting for the previous layer's xl STTs, and keep the HAM
clock-gate warm.

DMA plan: x is host-packed block-major ([P, NBLK, KC, NB] bf16, 4KB lines;
block 0 split per chunk) on the Sync queue; weights are host-packed
layer-major ([vw|cw|uw] per layer) with layer 0 split so the first pair's V
weights land first (Scalar queue for layer 0, GpSimd queue for the rest);
outputs store per-chunk from the Sync queue so the last store is small.
"""

import numpy as np
import ml_dtypes

import concourse.bacc as bacc
import concourse.tile as tile
from concourse import mybir
from concourse.bass_utils import run_bass_kernel_spmd

L, E, D, R, B = 3, 4, 512, 64, 16384
NCORES = 8
BC = B // NCORES          # batch columns per core (2048)
NB = 512                  # block of batch columns (one PSUM bank at fp32)
NBLK = BC // NB           # blocks per core
P = 128                   # partitions
KC = D // P               # contraction chunks over D (4)
NPAIR = E // 2            # expert pairs (2)

F32 = mybir.dt.float32
BF16 = mybir.dt.bfloat16
bf16 = ml_dtypes.bfloat16

# per-layer weight region: [vw (NPAIR*KC*P) | cw (NPAIR*P) | uw (NPAIR*D)]
VW_L = NPAIR * KC * P     # 1024
CW_L = NPAIR * P          # 256
UW_L = NPAIR * D          # 1024
WL_COLS = VW_L + CW_L + UW_L   # 2304
WB_COLS = L * WL_COLS          # 6912

_CACHE = {}


def _build_nc(bc=BC):
    """Build the per-core Bass/Tile kernel. Identical NEFF on all cores."""
    nblk = bc // NB
    nc = bacc.Bacc("TRN2", target_bir_lowering=False, debug=False,
                   num_devices=NCORES)

    xq_d = nc.dram_tensor("xq", [P, nblk, KC, NB], BF16, kind="ExternalInput")
    wb_d = nc.dram_tensor("wb", [P, WB_COLS], BF16, kind="ExternalInput")
    cb_d = nc.dram_tensor("cb", [P, L, KC], F32, kind="ExternalInput")
    oq_d = nc.dram_tensor("oq", [P, nblk, KC, NB], F32, kind="ExternalOutput")

    Tanh = mybir.ActivationFunctionType.Tanh
    ADD = mybir.AluOpType.add
    MULT = mybir.AluOpType.mult

    with tile.TileContext(nc) as tc:
        with (
            tc.tile_pool(name="wpool", bufs=1) as wpool,
            tc.tile_pool(name="xpool", bufs=1) as xpool,
            tc.tile_pool(name="xl_pool", bufs=10) as xl_pool,
            tc.tile_pool(name="vt_pool", bufs=6) as vt_pool,
            tc.tile_pool(name="cvt_pool", bufs=6) as cvt_pool,
            tc.tile_pool(name="out_pool", bufs=2) as out_pool,
            tc.tile_pool(name="psum_s", bufs=4, space="PSUM") as psum_s,
            tc.tile_pool(name="psum_t", bufs=4, space="PSUM") as psum_t,
        ):
            # ---- persistent inputs across three parallel DMA queues. The
            # very first pieces the PE needs (x block 0 chunk 0, vw layer 0
            # pair 0) get their own small DMAs so the first matmul is gated
            # on ~0.5MB, not the whole 3.8MB input set.
            xq_s = xpool.tile([P, nblk, KC, NB], BF16)
            wb_s = wpool.tile([P, WB_COLS], BF16)
            cb_s = wpool.tile([P, L, KC], F32)
            for k in range(KC):
                nc.sync.dma_start(xq_s[:, 0, k], xq_d[:, 0, k])
            HV = VW_L // 2
            nc.scalar.dma_start(wb_s[:, 0:HV], wb_d[:, 0:HV])
            nc.scalar.dma_start(wb_s[:, HV:VW_L], wb_d[:, HV:VW_L])
            nc.scalar.dma_start(wb_s[:, VW_L:WL_COLS], wb_d[:, VW_L:WL_COLS])
            nc.gpsimd.dma_start(cb_s[:], cb_d[:])
            nc.gpsimd.dma_start(wb_s[:, WL_COLS:2 * WL_COLS],
                                wb_d[:, WL_COLS:2 * WL_COLS])
            nc.gpsimd.dma_start(wb_s[:, 2 * WL_COLS:],
                                wb_d[:, 2 * WL_COLS:])
            for b in range(1, nblk):
                nc.sync.dma_start(xq_s[:, b], xq_d[:, b])

            def wv(l):
                base = l * WL_COLS
                return wb_s[:, base:base + VW_L].rearrange(
                    "p (q k m) -> p q k m", q=NPAIR, k=KC)

            def wc(l):
                base = l * WL_COLS + VW_L
                return wb_s[:, base:base + CW_L].rearrange(
                    "p (q m) -> p q m", q=NPAIR)

            def wu(l):
                base = l * WL_COLS + VW_L + CW_L
                return wb_s[:, base:base + UW_L].rearrange(
                    "p (q m) -> p q m", q=NPAIR)

            def chain_gen(b, l, xl):
                """v/cv/tanh chain for (b, l): yields after each piece so an
                outer chain can pump it at its fill sites. Final value (via
                StopIteration) is the cvt pair. Pair-major v so the first
                tanh starts as early as possible."""
                vps, vts, cvts = [], [], []
                for p in range(NPAIR):
                    vps.append(psum_t.tile([P, NB], F32,
                                           name=f"vp_{b}_{l}_{p}", tag="pt"))
                for p in range(NPAIR):
                    for k in range(KC):
                        nc.tensor.matmul(vps[p][:], wv(l)[:, p, k], xl[k],
                                         start=(k == 0), stop=(k == KC - 1))
                        if k == 1:
                            yield
                    vt = vt_pool.tile([P, NB], BF16,
                                      name=f"vt_{b}_{l}_{p}", tag="vt")
                    nc.scalar.activation(vt[:], vps[p][:], Tanh)
                    vts.append(vt)
                    yield
                for p in range(NPAIR):
                    cps = psum_t.tile([P, NB], F32, name=f"cp_{b}_{l}_{p}",
                                      tag="pt")
                    nc.tensor.matmul(cps[:], wc(l)[:, p], vts[p][:],
                                     start=True, stop=True)
                    cvt = cvt_pool.tile([P, NB], BF16,
                                        name=f"cvt_{b}_{l}_{p}", tag="cvt")
                    nc.scalar.activation(cvt[:], cps[:], Tanh)
                    cvts.append(cvt)
                    if p == 0:
                        yield
                return cvts

            class GenDriver:
                """Wraps a chain generator; captures its return value."""

                def __init__(self, gen):
                    self.gen = gen
                    self.done = False
                    self.value = None

                def pump(self):
                    if not self.done:
                        try:
                            next(self.gen)
                        except StopIteration as e:
                            self.done = True
                            self.value = e.value

                def finish(self):
                    while not self.done:
                        self.pump()
                    return self.value

            def run_chain(gen, fill=None, max_pumps=2):
                """Drive `gen` to completion, pumping `fill` at the first
                `max_pumps` yield sites. Unpumped fill pieces are reserved
                for the block boundary (fill.finish() there)."""
                pumps = 0
                while True:
                    try:
                        next(gen)
                    except StopIteration as e:
                        return e.value
                    if fill is not None and pumps < max_pumps:
                        fill.pump()
                        pumps += 1

            # u-step + xl/out production for (b, l).
            def u_and_x(b, l, cvts, s_tiles):
                uorder = [(0, 0), (1, 0), (0, 1), (1, 1),
                          (2, 0), (2, 1), (3, 0), (3, 1)]
                for m, p in uorder:
                    nc.tensor.matmul(
                        s_tiles[m],
                        wu(l)[:, p, m * P:(m + 1) * P],
                        cvts[p][:],
                        start=(l == 0 and p == 0),
                        stop=(l == 0 and p == 1),
                        skip_group_check=(l > 0),
                    )
                if l < L - 1:
                    nxt = []
                    for m in range(KC):
                        xln = xl_pool.tile([P, NB], BF16,
                                           name=f"xl_{b}_{l}_{m}", tag="xl")
                        nc.vector.scalar_tensor_tensor(
                            xln[:], s_tiles[m], cb_s[:, l, m:m + 1],
                            xq_s[:, b, m], ADD, MULT)
                        nxt.append(xln)
                    return [t[:] for t in nxt]
                ot = out_pool.tile([P, KC, NB], F32, name=f"ot_{b}", tag="ot")
                for m in range(KC):
                    nc.vector.scalar_tensor_tensor(
                        ot[:, m, :], s_tiles[m], cb_s[:, l, m:m + 1],
                        xq_s[:, b, m], ADD, MULT)
                    nc.sync.dma_start(oq_d[:, b, m], ot[:, m, :])
                return None

            # ---- software-pipelined main loop: the (b+1, l0) chain is
            # pumped piecewise at the fill sites of block b's l1/l2 chains.
            cvts = run_chain(chain_gen(0, 0, [xq_s[:, 0, k] for k in range(KC)]))
            fill = None
            for b in range(nblk):
                s_tiles = [psum_s.tile([P, NB], F32, name=f"s_{b}_{m}", tag="s")
                           for m in range(KC)]
                for l in range(L):
                    xl = u_and_x(b, l, cvts, s_tiles)
                    if l < L - 1:
                        if l == 0 and b + 1 < nblk:
                            fill = GenDriver(chain_gen(
                                b + 1, 0, [xq_s[:, b + 1, k] for k in range(KC)]))
                        cvts = run_chain(chain_gen(b, l + 1, xl), fill)
                    elif fill is not None:
                        cvts = fill.finish()
                        fill = None

    nc.compile()
    return nc


def _prep_weights(U, V, C, bias):
    """Host-side packing into the exact SBUF layouts (see module docstring)."""
    wb = np.empty([P, WB_COLS], dtype=bf16)
    for l in range(L):
        base = l * WL_COLS
        VwH = np.empty([P, NPAIR, KC, P], dtype=bf16)
        CwH = np.zeros([P, NPAIR, P], dtype=bf16)
        UwH = np.empty([P, NPAIR, D], dtype=bf16)
        for p in range(NPAIR):
            vpair = np.concatenate([V[l, 2 * p], V[l, 2 * p + 1]], axis=1)  # [D,128]
            VwH[:, p, :, :] = vpair.reshape(KC, P, P).transpose(1, 0, 2)
            upair = np.concatenate([U[l, 2 * p].T, U[l, 2 * p + 1].T], axis=0)  # [128,D]
            UwH[:, p, :] = upair
            CwH[:R, p, :R] = C[l, 2 * p]
            CwH[R:, p, R:] = C[l, 2 * p + 1]
        wb[:, base:base + VW_L] = VwH.reshape(P, VW_L)
        wb[:, base + VW_L:base + VW_L + CW_L] = CwH.reshape(P, CW_L)
        wb[:, base + VW_L + CW_L:base + WL_COLS] = UwH.reshape(P, UW_L)
    cb = 1.0 + E * np.cumsum(bias.astype(np.float32), axis=0)       # [L, D]
    cbH = np.ascontiguousarray(
        cb.reshape(L, KC, P).transpose(2, 0, 1)).astype(np.float32)  # [P, L, KC]
    return np.ascontiguousarray(wb), cbH


def _make_in_maps(x, U, V, C, G, bias):
    wbH, cbH = _prep_weights(np.asarray(U, np.float32),
                             np.asarray(V, np.float32),
                             np.asarray(C, np.float32),
                             np.asarray(bias, np.float32))
    # xq[core][p, b, k, j] = x[core*BC + b*NB + j, k*128 + p]  (bf16)
    xT = np.asarray(x, np.float32).T.astype(bf16)       # [D, B]
    xq = xT.reshape(KC, P, NCORES, NBLK, NB).transpose(2, 1, 3, 0, 4)
    in_maps = []
    for c in range(NCORES):
        in_maps.append({
            "xq": np.ascontiguousarray(xq[c]),
            "wb": wbH, "cb": cbH,
        })
    return in_maps


def _run(inputs, trace=False, **kw):
    key = "nc"
    if key not in _CACHE:
        _CACHE[key] = _build_nc()
    nc = _CACHE[key]
    in_maps = _make_in_maps(**inputs)
    res = run_bass_kernel_spmd(nc, in_maps, core_ids=list(range(NCORES)),
                               trace=trace, **kw)
    # oq[core][p, b, m, j] -> out[core*BC + b*NB + j, m*128 + p]
    out = np.empty((B, D), np.float32)
    for c in range(NCORES):
        oq = res.results[c]["oq"]                        # [P, NBLK, KC, NB]
        out[c * BC:(c + 1) * BC, :] = (
            oq.transpose(1, 3, 2, 0).reshape(BC, D))
    return out, res


def kernel(**inputs) -> np.ndarray:
    out, _ = _run(inputs, trace=False)
    return out


# revision 20
# speedup vs baseline: 1.0751x; 1.0751x over previous
"""Trainium2 Bass kernel for the mixed low-rank-expert DCN-v2 block (nn_DCN_51539607711).

Reference math (L=3 layers, E=4 experts, D=512, R=64, B=16384):
  x_{l+1} = sum_e x0 * (tanh(tanh(x_l V_e) C_e) U_e^T + b_l) * gate_e + x_l
The gate is softmax over a size-1 axis == exactly 1.0, so G never affects the
output. With gate == 1 the recurrence telescopes:
  x_{l+1} = x0 * (1 + sum_{i<=l} (A_i(x_i) + E*b_i)),
  A_i(x) = sum_e U_e tanh(C_e^T tanh(V_e^T x))
so the residual stream is carried as a single PSUM accumulator s = sum_i A_i
(fp32, accumulated by the PE across all experts AND layers), and each layer's
activation update is ONE fused DVE op per chunk:
  x_{l+1} = (s + c_l[d]) * x0,   c_l = 1 + E*cumsum(b)_l   (per-partition scalar)

Distribution: pure data-parallel over B across 8 cores (2048 rows/core),
weights replicated. Activations live feature-major (xT: [D, B]); every matmul
contracts on the partition dim with zero on-device transposes. Experts are
packed in pairs to fill all 128 partitions/output rows:
  v-step :  vT[pair]  = Vpair^T  @ xlT    (lhsT = Vpair [D,128], K=D in 4 chunks)
  cv-step:  cvT[pair] = blockdiag(C_e0,C_e1)^T @ vT[pair]   (K=128)
  ucv    :  s[mchunk] += Upair^T-packed @ cvT[pair]          (K=128, accum)
All matmul operands bf16 (fp32 PSUM accumulation); x0 is kept only in bf16
(rel-err budget is 2e-2; bf16 x0 costs ~3e-3).

Scheduling: the v/cv/tanh chain of block b+1's layer 0 depends only on the x
DMA, so it is emitted as a generator pumped at fill sites inside block b's
layer-1/2 chains. Those fill matmuls occupy the PE exactly where it would
otherwise stall waiting for the previous layer's xl STTs, and keep the HAM
clock-gate warm.

DMA plan: x is host-packed block-major ([P, NBLK, KC, NB] bf16, 4KB lines;
block 0 split per chunk) on the Sync queue; weights are host-packed
layer-major ([vw|cw|uw] per layer) with layer 0 split so the first pair's V
weights land first (Scalar queue for layer 0, GpSimd queue for the rest);
outputs store per-chunk from the Sync queue so the last store is small.
"""

import numpy as np
import ml_dtypes

import concourse.bacc as bacc
import concourse.tile as tile
from concourse import mybir
from concourse.bass_utils import run_bass_kernel_spmd

L, E, D, R, B = 3, 4, 512, 64, 16384
NCORES = 8
BC = B // NCORES          # batch columns per core (2048)
NB = 512                  # block of batch columns (one PSUM bank at fp32)
NBLK = BC // NB           # blocks per core
P = 128                   # partitions
KC = D // P               # contraction chunks over D (4)
NPAIR = E // 2            # expert pairs (2)

F32 = mybir.dt.float32
BF16 = mybir.dt.bfloat16
bf16 = ml_dtypes.bfloat16

# per-layer weight region: [vw (NPAIR*KC*P) | cw (NPAIR*P) | uw (NPAIR*D)]
VW_L = NPAIR * KC * P     # 1024
CW_L = NPAIR * P          # 256
UW_L = NPAIR * D          # 1024
WL_COLS = VW_L + CW_L + UW_L   # 2304
WB_COLS = L * WL_COLS          # 6912

_CACHE = {}


def _build_nc(bc=BC):
    """Build the per-core Bass/Tile kernel. Identical NEFF on all cores."""
    nblk = bc // NB
    nc = bacc.Bacc("TRN2", target_bir_lowering=False, debug=False,
                   num_devices=NCORES)

    xq_d = nc.dram_tensor("xq", [P, nblk, KC, NB], BF16, kind="ExternalInput")
    wb_d = nc.dram_tensor("wb", [P, WB_COLS], BF16, kind="ExternalInput")
    cb_d = nc.dram_tensor("cb", [P, L, KC], F32, kind="ExternalInput")
    oq_d = nc.dram_tensor("oq", [P, nblk, KC, NB], F32, kind="ExternalOutput")

    Tanh = mybir.ActivationFunctionType.Tanh
    ADD = mybir.AluOpType.add
    MULT = mybir.AluOpType.mult

    with tile.TileContext(nc) as tc:
        with (
            tc.tile_pool(name="wpool", bufs=1) as wpool,
            tc.tile_pool(name="xpool", bufs=1) as xpool,
            tc.tile_pool(name="xl_pool", bufs=10) as xl_pool,
            tc.tile_pool(name="vt_pool", bufs=6) as vt_pool,
            tc.tile_pool(name="cvt_pool", bufs=6) as cvt_pool,
            tc.tile_pool(name="out_pool", bufs=2) as out_pool,
            tc.tile_pool(name="psum_s", bufs=4, space="PSUM") as psum_s,
            tc.tile_pool(name="psum_t", bufs=4, space="PSUM") as psum_t,
        ):
            # ---- persistent inputs across three parallel DMA queues. The
            # very first pieces the PE needs (x block 0 chunk 0, vw layer 0
            # pair 0) get their own small DMAs so the first matmul is gated
            # on ~0.5MB, not the whole 3.8MB input set.
            xq_s = xpool.tile([P, nblk, KC, NB], BF16)
            wb_s = wpool.tile([P, WB_COLS], BF16)
            cb_s = wpool.tile([P, L, KC], F32)
            nc.sync.dma_start(xq_s[:, 0], xq_d[:, 0])
            nc.scalar.dma_start(wb_s[:, 0:VW_L], wb_d[:, 0:VW_L])
            nc.scalar.dma_start(wb_s[:, VW_L:WL_COLS], wb_d[:, VW_L:WL_COLS])
            nc.gpsimd.dma_start(cb_s[:], cb_d[:])
            nc.gpsimd.dma_start(wb_s[:, WL_COLS:2 * WL_COLS],
                                wb_d[:, WL_COLS:2 * WL_COLS])
            nc.gpsimd.dma_start(wb_s[:, 2 * WL_COLS:],
                                wb_d[:, 2 * WL_COLS:])
            for b in range(1, nblk):
                nc.sync.dma_start(xq_s[:, b], xq_d[:, b])

            def wv(l):
                base = l * WL_COLS
                return wb_s[:, base:base + VW_L].rearrange(
                    "p (q k m) -> p q k m", q=NPAIR, k=KC)

            def wc(l):
                base = l * WL_COLS + VW_L
                return wb_s[:, base:base + CW_L].rearrange(
                    "p (q m) -> p q m", q=NPAIR)

            def wu(l):
                base = l * WL_COLS + VW_L + CW_L
                return wb_s[:, base:base + UW_L].rearrange(
                    "p (q m) -> p q m", q=NPAIR)

            def chain_gen(b, l, xl):
                """v/cv/tanh chain for (b, l): yields after each piece so an
                outer chain can pump it at its fill sites. Final value (via
                StopIteration) is the cvt pair. Pair-major v so the first
                tanh starts as early as possible."""
                vps, vts, cvts = [], [], []
                for p in range(NPAIR):
                    vps.append(psum_t.tile([P, NB], F32,
                                           name=f"vp_{b}_{l}_{p}", tag="pt"))
                for p in range(NPAIR):
                    for k in range(KC):
                        nc.tensor.matmul(vps[p][:], wv(l)[:, p, k], xl[k],
                                         start=(k == 0), stop=(k == KC - 1))
                        if k == KC - 1:
                            vt = vt_pool.tile([P, NB], BF16,
                                              name=f"vt_{b}_{l}_{p}", tag="vt")
                            nc.scalar.activation(vt[:], vps[p][:], Tanh)
                            vts.append(vt)
                        yield
                for p in range(NPAIR):
                    cps = psum_t.tile([P, NB], F32, name=f"cp_{b}_{l}_{p}",
                                      tag="pt")
                    nc.tensor.matmul(cps[:], wc(l)[:, p], vts[p][:],
                                     start=True, stop=True)
                    cvt = cvt_pool.tile([P, NB], BF16,
                                        name=f"cvt_{b}_{l}_{p}", tag="cvt")
                    nc.scalar.activation(cvt[:], cps[:], Tanh)
                    cvts.append(cvt)
                    if p == 0:
                        yield
                return cvts

            class GenDriver:
                """Wraps a chain generator; captures its return value."""

                def __init__(self, gen):
                    self.gen = gen
                    self.done = False
                    self.value = None

                def pump(self):
                    if not self.done:
                        try:
                            next(self.gen)
                        except StopIteration as e:
                            self.done = True
                            self.value = e.value

                def finish(self):
                    while not self.done:
                        self.pump()
                    return self.value

            def run_chain(gen, fill=None, pump_at={7: 2, 8: 1}):
                """Drive `gen` to completion, pumping `fill` at the yield
                sites where the PE would otherwise stall (before the cv
                matmuls, which wait on the tanh chain). Unpumped fill pieces
                are reserved for the pre-u site and the block boundary."""
                idx = 0
                while True:
                    try:
                        next(gen)
                    except StopIteration as e:
                        return e.value
                    if fill is not None:
                        for _ in range(pump_at.get(idx, 0)):
                            fill.pump()
                    idx += 1

            # u-step + xl/out production for (b, l).
            def u_and_x(b, l, cvts, s_tiles):
                uorder = [(0, 0), (1, 0), (0, 1), (1, 1),
                          (2, 0), (2, 1), (3, 0), (3, 1)]
                for m, p in uorder:
                    nc.tensor.matmul(
                        s_tiles[m],
                        wu(l)[:, p, m * P:(m + 1) * P],
                        cvts[p][:],
                        start=(l == 0 and p == 0),
                        stop=(l == 0 and p == 1),
                        skip_group_check=(l > 0),
                    )
                if l < L - 1:
                    nxt = []
                    for m in range(KC):
                        xln = xl_pool.tile([P, NB], BF16,
                                           name=f"xl_{b}_{l}_{m}", tag="xl")
                        nc.vector.scalar_tensor_tensor(
                            xln[:], s_tiles[m], cb_s[:, l, m:m + 1],
                            xq_s[:, b, m], ADD, MULT)
                        nxt.append(xln)
                    return [t[:] for t in nxt]
                ot = out_pool.tile([P, KC, NB], F32, name=f"ot_{b}", tag="ot")
                for m in range(KC):
                    nc.vector.scalar_tensor_tensor(
                        ot[:, m, :], s_tiles[m], cb_s[:, l, m:m + 1],
                        xq_s[:, b, m], ADD, MULT)
                    nc.sync.dma_start(oq_d[:, b, m], ot[:, m, :])
                return None

            # ---- software-pipelined main loop: the (b+1, l0) chain is
            # pumped piecewise at the fill sites of block b's l1/l2 chains.
            cvts = run_chain(chain_gen(0, 0, [xq_s[:, 0, k] for k in range(KC)]))
            fill = None
            for b in range(nblk):
                s_tiles = [psum_s.tile([P, NB], F32, name=f"s_{b}_{m}", tag="s")
                           for m in range(KC)]
                for l in range(L):
                    if l > 0 and fill is not None:
                        fill.pump()            # pre-u stall site
                    xl = u_and_x(b, l, cvts, s_tiles)
                    if l < L - 1:
                        if l == 0 and b + 1 < nblk:
                            fill = GenDriver(chain_gen(
                                b + 1, 0, [xq_s[:, b + 1, k] for k in range(KC)]))
                        cvts = run_chain(chain_gen(b, l + 1, xl), fill)
                    elif fill is not None:
                        cvts = fill.finish()
                        fill = None

    nc.compile()
    return nc


def _prep_weights(U, V, C, bias):
    """Host-side packing into the exact SBUF layouts (see module docstring)."""
    wb = np.empty([P, WB_COLS], dtype=bf16)
    for l in range(L):
        base = l * WL_COLS
        VwH = np.empty([P, NPAIR, KC, P], dtype=bf16)
        CwH = np.zeros([P, NPAIR, P], dtype=bf16)
        UwH = np.empty([P, NPAIR, D], dtype=bf16)
        for p in range(NPAIR):
            vpair = np.concatenate([V[l, 2 * p], V[l, 2 * p + 1]], axis=1)  # [D,128]
            VwH[:, p, :, :] = vpair.reshape(KC, P, P).transpose(1, 0, 2)
            upair = np.concatenate([U[l, 2 * p].T, U[l, 2 * p + 1].T], axis=0)  # [128,D]
            UwH[:, p, :] = upair
            CwH[:R, p, :R] = C[l, 2 * p]
            CwH[R:, p, R:] = C[l, 2 * p + 1]
        wb[:, base:base + VW_L] = VwH.reshape(P, VW_L)
        wb[:, base + VW_L:base + VW_L + CW_L] = CwH.reshape(P, CW_L)
        wb[:, base + VW_L + CW_L:base + WL_COLS] = UwH.reshape(P, UW_L)
    cb = 1.0 + E * np.cumsum(bias.astype(np.float32), axis=0)       # [L, D]
    cbH = np.ascontiguousarray(
        cb.reshape(L, KC, P).transpose(2, 0, 1)).astype(np.float32)  # [P, L, KC]
    return np.ascontiguousarray(wb), cbH


def _make_in_maps(x, U, V, C, G, bias):
    wbH, cbH = _prep_weights(np.asarray(U, np.float32),
                             np.asarray(V, np.float32),
                             np.asarray(C, np.float32),
                             np.asarray(bias, np.float32))
    # xq[core][p, b, k, j] = x[core*BC + b*NB + j, k*128 + p]  (bf16)
    xT = np.asarray(x, np.float32).T.astype(bf16)       # [D, B]
    xq = xT.reshape(KC, P, NCORES, NBLK, NB).transpose(2, 1, 3, 0, 4)
    in_maps = []
    for c in range(NCORES):
        in_maps.append({
            "xq": np.ascontiguousarray(xq[c]),
            "wb": wbH, "cb": cbH,
        })
    return in_maps


def _run(inputs, trace=False, **kw):
    key = "nc"
    if key not in _CACHE:
        _CACHE[key] = _build_nc()
    nc = _CACHE[key]
    in_maps = _make_in_maps(**inputs)
    res = run_bass_kernel_spmd(nc, in_maps, core_ids=list(range(NCORES)),
                               trace=trace, **kw)
    # oq[core][p, b, m, j] -> out[core*BC + b*NB + j, m*128 + p]
    out = np.empty((B, D), np.float32)
    for c in range(NCORES):
        oq = res.results[c]["oq"]                        # [P, NBLK, KC, NB]
        out[c * BC:(c + 1) * BC, :] = (
            oq.transpose(1, 3, 2, 0).reshape(BC, D))
    return out, res


def kernel(**inputs) -> np.ndarray:
    out, _ = _run(inputs, trace=False)
    return out


# revision 21
# speedup vs baseline: 1.0995x; 1.0227x over previous
"""Trainium2 Bass kernel for the mixed low-rank-expert DCN-v2 block (nn_DCN_51539607711).

Reference math (L=3 layers, E=4 experts, D=512, R=64, B=16384):
  x_{l+1} = sum_e x0 * (tanh(tanh(x_l V_e) C_e) U_e^T + b_l) * gate_e + x_l
The gate is softmax over a size-1 axis == exactly 1.0, so G never affects the
output. With gate == 1 the recurrence telescopes:
  x_{l+1} = x0 * (1 + sum_{i<=l} (A_i(x_i) + E*b_i)),
  A_i(x) = sum_e U_e tanh(C_e^T tanh(V_e^T x))
so the residual stream is carried as a single PSUM accumulator s = sum_i A_i
(fp32, accumulated by the PE across all experts AND layers), and each layer's
activation update is ONE fused DVE op per chunk:
  x_{l+1} = (s + c_l[d]) * x0,   c_l = 1 + E*cumsum(b)_l   (per-partition scalar)

Distribution: pure data-parallel over B across 8 cores (2048 rows/core),
weights replicated. Activations live feature-major (xT: [D, B]); every matmul
contracts on the partition dim with zero on-device transposes. Experts are
packed in pairs to fill all 128 partitions/output rows:
  v-step :  vT[pair]  = Vpair^T  @ xlT    (lhsT = Vpair [D,128], K=D in 4 chunks)
  cv-step:  cvT[pair] = blockdiag(C_e0,C_e1)^T @ vT[pair]   (K=128)
  ucv    :  s[mchunk] += Upair^T-packed @ cvT[pair]          (K=128, accum)
All matmul operands bf16 (fp32 PSUM accumulation); x0 is kept only in bf16
(rel-err budget is 2e-2; bf16 x0 costs ~3e-3).

Scheduling: the v/cv/tanh chain of block b+1's layer 0 depends only on the x
DMA, so it is emitted as a generator pumped at fill sites inside block b's
layer-1/2 chains. Those fill matmuls occupy the PE exactly where it would
otherwise stall waiting for the previous layer's xl STTs, and keep the HAM
clock-gate warm.

DMA plan: x is host-packed block-major ([P, NBLK, KC, NB] bf16, 4KB lines;
block 0 split per chunk) on the Sync queue; weights are host-packed
layer-major ([vw|cw|uw] per layer) with layer 0 split so the first pair's V
weights land first (Scalar queue for layer 0, GpSimd queue for the rest);
outputs store per-chunk from the Sync queue so the last store is small.
"""

import numpy as np
import ml_dtypes

import concourse.bacc as bacc
import concourse.tile as tile
from concourse import mybir
from concourse.bass_utils import run_bass_kernel_spmd

L, E, D, R, B = 3, 4, 512, 64, 16384
NCORES = 8
BC = B // NCORES          # batch columns per core (2048)
NB = 512                  # block of batch columns (one PSUM bank at fp32)
NBLK = BC // NB           # blocks per core
P = 128                   # partitions
KC = D // P               # contraction chunks over D (4)
NPAIR = E // 2            # expert pairs (2)

F32 = mybir.dt.float32
BF16 = mybir.dt.bfloat16
bf16 = ml_dtypes.bfloat16

# per-layer weight region: [vw (NPAIR*KC*P) | cw (NPAIR*P) | uw (NPAIR*D)]
VW_L = NPAIR * KC * P     # 1024
CW_L = NPAIR * P          # 256
UW_L = NPAIR * D          # 1024
WL_COLS = VW_L + CW_L + UW_L   # 2304
WB_COLS = L * WL_COLS          # 6912

_CACHE = {}


def _build_nc(bc=BC):
    """Build the per-core Bass/Tile kernel. Identical NEFF on all cores."""
    nblk = bc // NB
    nc = bacc.Bacc("TRN2", target_bir_lowering=False, debug=False,
                   num_devices=NCORES)

    xq_d = nc.dram_tensor("xq", [P, nblk, KC, NB], BF16, kind="ExternalInput")
    wb_d = nc.dram_tensor("wb", [P, WB_COLS], BF16, kind="ExternalInput")
    cb_d = nc.dram_tensor("cb", [P, L, KC], F32, kind="ExternalInput")
    oq_d = nc.dram_tensor("oq", [P, nblk, KC, NB], F32, kind="ExternalOutput")

    Tanh = mybir.ActivationFunctionType.Tanh
    ADD = mybir.AluOpType.add
    MULT = mybir.AluOpType.mult

    with tile.TileContext(nc) as tc:
        with (
            tc.tile_pool(name="wpool", bufs=1) as wpool,
            tc.tile_pool(name="xpool", bufs=1) as xpool,
            tc.tile_pool(name="xl_pool", bufs=10) as xl_pool,
            tc.tile_pool(name="vt_pool", bufs=6) as vt_pool,
            tc.tile_pool(name="cvt_pool", bufs=6) as cvt_pool,
            tc.tile_pool(name="out_pool", bufs=2) as out_pool,
            tc.tile_pool(name="psum_s", bufs=4, space="PSUM") as psum_s,
            tc.tile_pool(name="psum_to", bufs=2, space="PSUM") as psum_to,
            tc.tile_pool(name="psum_tf", bufs=2, space="PSUM") as psum_tf,
        ):
            # ---- persistent inputs across three parallel DMA queues. The
            # very first pieces the PE needs (x block 0 chunk 0, vw layer 0
            # pair 0) get their own small DMAs so the first matmul is gated
            # on ~0.5MB, not the whole 3.8MB input set.
            xq_s = xpool.tile([P, nblk, KC, NB], BF16)
            wb_s = wpool.tile([P, WB_COLS], BF16)
            cb_s = wpool.tile([P, L, KC], F32)
            nc.sync.dma_start(xq_s[:, 0, 0:2], xq_d[:, 0, 0:2])
            nc.sync.dma_start(xq_s[:, 0, 2:4], xq_d[:, 0, 2:4])
            nc.scalar.dma_start(wb_s[:, 0:VW_L], wb_d[:, 0:VW_L])
            nc.scalar.dma_start(wb_s[:, VW_L:WL_COLS], wb_d[:, VW_L:WL_COLS])
            nc.gpsimd.dma_start(cb_s[:], cb_d[:])
            nc.gpsimd.dma_start(wb_s[:, WL_COLS:2 * WL_COLS],
                                wb_d[:, WL_COLS:2 * WL_COLS])
            nc.gpsimd.dma_start(wb_s[:, 2 * WL_COLS:],
                                wb_d[:, 2 * WL_COLS:])
            for b in range(1, nblk):
                nc.sync.dma_start(xq_s[:, b], xq_d[:, b])

            def wv(l):
                base = l * WL_COLS
                return wb_s[:, base:base + VW_L].rearrange(
                    "p (q k m) -> p q k m", q=NPAIR, k=KC)

            def wc(l):
                base = l * WL_COLS + VW_L
                return wb_s[:, base:base + CW_L].rearrange(
                    "p (q m) -> p q m", q=NPAIR)

            def wu(l):
                base = l * WL_COLS + VW_L + CW_L
                return wb_s[:, base:base + UW_L].rearrange(
                    "p (q m) -> p q m", q=NPAIR)

            def chain_gen(b, l, xl, pool):
                """v/cv/tanh chain for (b, l): yields after each piece so an
                outer chain can pump it at its fill sites. Final value (via
                StopIteration) is the cvt pair. Pair-major v so the first
                tanh starts as early as possible."""
                vps, vts, cvts = [], [], []
                for p in range(NPAIR):
                    vps.append(pool.tile([P, NB], F32,
                                         name=f"vp_{b}_{l}_{p}", tag="pt"))
                for p in range(NPAIR):
                    for k in range(KC):
                        nc.tensor.matmul(vps[p][:], wv(l)[:, p, k], xl[k],
                                         start=(k == 0), stop=(k == KC - 1))
                        if k == KC - 1:
                            vt = vt_pool.tile([P, NB], BF16,
                                              name=f"vt_{b}_{l}_{p}", tag="vt")
                            nc.scalar.activation(vt[:], vps[p][:], Tanh)
                            vts.append(vt)
                        yield
                for p in range(NPAIR):
                    cps = pool.tile([P, NB], F32, name=f"cp_{b}_{l}_{p}",
                                    tag="pt")
                    nc.tensor.matmul(cps[:], wc(l)[:, p], vts[p][:],
                                     start=True, stop=True)
                    cvt = cvt_pool.tile([P, NB], BF16,
                                        name=f"cvt_{b}_{l}_{p}", tag="cvt")
                    nc.scalar.activation(cvt[:], cps[:], Tanh)
                    cvts.append(cvt)
                    if p == 0:
                        yield
                return cvts

            class GenDriver:
                """Wraps a chain generator; captures its return value."""

                def __init__(self, gen):
                    self.gen = gen
                    self.done = False
                    self.value = None

                def pump(self):
                    if not self.done:
                        try:
                            next(self.gen)
                        except StopIteration as e:
                            self.done = True
                            self.value = e.value

                def finish(self):
                    while not self.done:
                        self.pump()
                    return self.value

            def run_chain(gen, fill=None, pump_at={7: 2, 8: 1}):
                """Drive `gen` to completion, pumping `fill` at the yield
                sites where the PE would otherwise stall (before the cv
                matmuls, which wait on the tanh chain). Unpumped fill pieces
                are reserved for the pre-u site and the block boundary."""
                idx = 0
                while True:
                    try:
                        next(gen)
                    except StopIteration as e:
                        return e.value
                    if fill is not None:
                        for _ in range(pump_at.get(idx, 0)):
                            fill.pump()
                    idx += 1

            # u-step + xl/out production for (b, l).
            def u_and_x(b, l, cvts, s_tiles):
                uorder = [(0, 0), (1, 0), (0, 1), (1, 1),
                          (2, 0), (2, 1), (3, 0), (3, 1)]
                for m, p in uorder:
                    nc.tensor.matmul(
                        s_tiles[m],
                        wu(l)[:, p, m * P:(m + 1) * P],
                        cvts[p][:],
                        start=(l == 0 and p == 0),
                        stop=(l == 0 and p == 1),
                        skip_group_check=(l > 0),
                    )
                if l < L - 1:
                    nxt = []
                    for m in range(KC):
                        xln = xl_pool.tile([P, NB], BF16,
                                           name=f"xl_{b}_{l}_{m}", tag="xl")
                        nc.vector.scalar_tensor_tensor(
                            xln[:], s_tiles[m], cb_s[:, l, m:m + 1],
                            xq_s[:, b, m], ADD, MULT)
                        nxt.append(xln)
                    return [t[:] for t in nxt]
                ot = out_pool.tile([P, KC, NB], F32, name=f"ot_{b}", tag="ot")
                for m in range(KC):
                    nc.vector.scalar_tensor_tensor(
                        ot[:, m, :], s_tiles[m], cb_s[:, l, m:m + 1],
                        xq_s[:, b, m], ADD, MULT)
                    nc.sync.dma_start(oq_d[:, b, m], ot[:, m, :])
                return None

            # ---- software-pipelined main loop: the (b+1, l0) chain is
            # pumped piecewise at the fill sites of block b's l1/l2 chains.
            cvts = run_chain(chain_gen(0, 0, [xq_s[:, 0, k] for k in range(KC)],
                                       psum_to))
            fill = None
            for b in range(nblk):
                s_tiles = [psum_s.tile([P, NB], F32, name=f"s_{b}_{m}", tag="s")
                           for m in range(KC)]
                for l in range(L):
                    if l > 0 and fill is not None:
                        fill.pump()            # pre-u stall site
                    xl = u_and_x(b, l, cvts, s_tiles)
                    if l < L - 1:
                        if l == 0 and b + 1 < nblk:
                            fill = GenDriver(chain_gen(
                                b + 1, 0,
                                [xq_s[:, b + 1, k] for k in range(KC)],
                                psum_tf))
                        cvts = run_chain(chain_gen(b, l + 1, xl, psum_to), fill)
                    elif fill is not None:
                        cvts = fill.finish()
                        fill = None

    nc.compile()
    return nc


def _prep_weights(U, V, C, bias):
    """Host-side packing into the exact SBUF layouts (see module docstring)."""
    wb = np.empty([P, WB_COLS], dtype=bf16)
    for l in range(L):
        base = l * WL_COLS
        VwH = np.empty([P, NPAIR, KC, P], dtype=bf16)
        CwH = np.zeros([P, NPAIR, P], dtype=bf16)
        UwH = np.empty([P, NPAIR, D], dtype=bf16)
        for p in range(NPAIR):
            vpair = np.concatenate([V[l, 2 * p], V[l, 2 * p + 1]], axis=1)  # [D,128]
            VwH[:, p, :, :] = vpair.reshape(KC, P, P).transpose(1, 0, 2)
            upair = np.concatenate([U[l, 2 * p].T, U[l, 2 * p + 1].T], axis=0)  # [128,D]
            UwH[:, p, :] = upair
            CwH[:R, p, :R] = C[l, 2 * p]
            CwH[R:, p, R:] = C[l, 2 * p + 1]
        wb[:, base:base + VW_L] = VwH.reshape(P, VW_L)
        wb[:, base + VW_L:base + VW_L + CW_L] = CwH.reshape(P, CW_L)
        wb[:, base + VW_L + CW_L:base + WL_COLS] = UwH.reshape(P, UW_L)
    cb = 1.0 + E * np.cumsum(bias.astype(np.float32), axis=0)       # [L, D]
    cbH = np.ascontiguousarray(
        cb.reshape(L, KC, P).transpose(2, 0, 1)).astype(np.float32)  # [P, L, KC]
    return np.ascontiguousarray(wb), cbH


def _make_in_maps(x, U, V, C, G, bias):
    wbH, cbH = _prep_weights(np.asarray(U, np.float32),
                             np.asarray(V, np.float32),
                             np.asarray(C, np.float32),
                             np.asarray(bias, np.float32))
    # xq[core][p, b, k, j] = x[core*BC + b*NB + j, k*128 + p]  (bf16)
    xT = np.asarray(x, np.float32).T.astype(bf16)       # [D, B]
    xq = xT.reshape(KC, P, NCORES, NBLK, NB).transpose(2, 1, 3, 0, 4)
    in_maps = []
    for c in range(NCORES):
        in_maps.append({
            "xq": np.ascontiguousarray(xq[c]),
            "wb": wbH, "cb": cbH,
        })
    return in_maps


def _run(inputs, trace=False, **kw):
    key = "nc"
    if key not in _CACHE:
        _CACHE[key] = _build_nc()
    nc = _CACHE[key]
    in_maps = _make_in_maps(**inputs)
    res = run_bass_kernel_spmd(nc, in_maps, core_ids=list(range(NCORES)),
                               trace=trace, **kw)
    # oq[core][p, b, m, j] -> out[core*BC + b*NB + j, m*128 + p]
    out = np.empty((B, D), np.float32)
    for c in range(NCORES):
        oq = res.results[c]["oq"]                        # [P, NBLK, KC, NB]
        out[c * BC:(c + 1) * BC, :] = (
            oq.transpose(1, 3, 2, 0).reshape(BC, D))
    return out, res


def kernel(**inputs) -> np.ndarray:
    out, _ = _run(inputs, trace=False)
    return out
